# revision 10
# baseline (speedup 1.0000x reference)
"""GraphNorm-style segmented normalization on 8 Trainium2 NeuronCores.

Strategy (x:[500000,256] f32, batch sorted int, 4096 graphs, params [256]):

- Host: graphs sorted by size (descending), dealt round-robin to 8 cores;
  slot k on every core holds that core's rank-(8k+c) graph, padded to the
  canonical size S_k = size(rank 8k) (rounded to even). Slot structure is
  identical across cores -> one SPMD Bass program, per-core data.
- Host packs each core's nodes channel-major and HALF-INTERLEAVED:
  xt[p, 2*w + h] = x[node w, h*128 + p]. A single bn_stats over a slot's
  [128, 2*S] range yields independent stats for the lo channel half
  (even elements) and hi half (odd elements) -- one stats op per slot
  (the ISA caps BNStats at one range / 6 outputs per op).
- Device math (per slot, half h, channel c; m/v = bn mean / cnt*var):
    sigma'^2/w^2 = v*c3' + m^2*c4' (+EPS/w^2 via sqrt bias), with
    c3' = 1/(n*w^2), c4' = (S/n)*(1 + (a^2-2a)*S/n)/w^2 host constants.
    A = w*rstd = reciprocal(sqrt(.)), B = b - a*mu*A.
  This keeps only 6 TT + 1 recip per super on DVE; the small per-half
  affines (sqrt bias, B finish) ride ACT.
- Apply out = A*x + B per (slot, half): split across THREE engines (DVE
  tensor_scalar / ACT activation-Identity / GPSIMD tensor_scalar) by
  greedy cost balance, written to bf16 Y tiles (halves store traffic;
  DMA is the top bottleneck). Stores are 2-chunk (~1 MB) granules.
- Host un-interleaves, upcasts bf16 -> f32, scatters rows back.
"""
import sys

if "/opt/trn_rl_repo" not in sys.path:
    sys.path.insert(0, "/opt/trn_rl_repo")

import numpy as np

import concourse.bacc as bacc
import concourse.tile as tile
from concourse import mybir
from concourse.bass_utils import run_bass_kernel_spmd

F32 = mybir.dt.float32
BF16 = mybir.dt.bfloat16
EPS = 1e-9
N_CORES = 8
H = 256
MINI_TGT = 1024     # nodes per chunk (DMA/pipeline granule)
X_BUFS = 14         # X alive ~3 supers (applies lag fronts by 2)
Y_BUFS = 5
USE_GPSIMD = True
# measured per-op cost models (ns) for the apply split, S = slot size
DVE_APPLY_NS = lambda S: 184 + 1.042 * S
ACT_APPLY_NS = lambda S: 399 + 0.833 * S
GP_APPLY_NS = lambda S: 131 + 2.9 * S

_program_cache = {}
_last_run = None


def _plan_slots(sizes, n_cores):
    G = len(sizes)
    Gp = ((G + n_cores - 1) // n_cores) * n_cores
    sizes_p = np.concatenate([sizes, np.zeros(Gp - len(sizes), sizes.dtype)])
    order = np.argsort(-sizes_p, kind="stable")
    ranked = order.reshape(-1, n_cores)
    rank_sz = sizes_p[order].reshape(-1, n_cores)
    S = rank_sz[:, 0]
    keep = S > 0
    ranked = ranked[keep]
    S = S[keep].astype(np.int64)
    S = ((S + 1) // 2) * 2
    offs = np.concatenate([[0], np.cumsum(S)])
    M = len(S)
    chunks = []
    k0 = 0
    acc = 0
    for k in range(M):
        acc += int(S[k])
        if acc >= MINI_TGT:
            chunks.append((k0, k + 1))
            k0 = k + 1
            acc = 0
    if k0 < M:
        chunks.append((k0, M))
    return ranked, S, offs, chunks


def _plan_supers(minis):
    """Group chunks into supers of 4, but use supers of 2 for the first
    and last two groups: a shallower pipeline head fills faster and a
    shallower tail drains faster (applies lag fronts by 2 supers)."""
    n = len(minis)
    sizes = []
    head = [2, 2] if n >= 12 else []
    tail = [2, 2] if n >= 12 else []
    mid = n - sum(head) - sum(tail)
    sizes = head + [4] * (mid // 4)
    rem = mid % 4
    if rem:
        sizes.append(rem)
    sizes += tail
    out = []
    i = 0
    for s in sizes:
        out.append(minis[i:i + s])
        i += s
    return [s_ for s_ in out if s_]


def _build_program(S, offs, supers, M, Np, w_nonneg):
    nc = bacc.Bacc("TRN2", target_bir_lowering=False, debug=False,
                   num_devices=N_CORES)
    xt_d = nc.dram_tensor("xt", [128, 2 * Np], F32, kind="ExternalInput")
    c1_d = nc.dram_tensor("c1", [128, M, 2], F32, kind="ExternalInput")
    c3_d = nc.dram_tensor("c3", [128, M, 2], F32, kind="ExternalInput")
    c4_d = nc.dram_tensor("c4", [128, M, 2], F32, kind="ExternalInput")
    b_d = nc.dram_tensor("bp", [128, 2], F32, kind="ExternalInput")
    na_d = nc.dram_tensor("nap", [128, 2], F32, kind="ExternalInput")
    eps_d = nc.dram_tensor("epsp", [128, 2], F32, kind="ExternalInput")
    sw_d = nc.dram_tensor("swp", [128, 2], F32, kind="ExternalInput")
    yt_d = nc.dram_tensor("yt", [128, 2 * Np], BF16, kind="ExternalOutput")

    mult = mybir.AluOpType.mult
    add = mybir.AluOpType.add
    ident = mybir.ActivationFunctionType.Identity
    sqrtf = mybir.ActivationFunctionType.Sqrt

    with tile.TileContext(nc) as tc:
        with (
            tc.tile_pool(name="const", bufs=1) as constp,
            tc.tile_pool(name="xp", bufs=X_BUFS) as xp,
            tc.tile_pool(name="yp", bufs=Y_BUFS) as yp,
            tc.tile_pool(name="stp", bufs=2) as stp,
            tc.tile_pool(name="abp", bufs=2) as abp,
            tc.tile_pool(name="abp3", bufs=3) as abp3,
        ):
            c1t = constp.tile([128, M, 2], F32)
            c3t = constp.tile([128, M, 2], F32)
            c4t = constp.tile([128, M, 2], F32)
            bt = constp.tile([128, 2], F32)
            nat = constp.tile([128, 2], F32)
            epst = constp.tile([128, 2], F32)
            swt = constp.tile([128, 2], F32)
            nc.sync.dma_start(c1t[:], c1_d[:, :, :])
            nc.sync.dma_start(c3t[:], c3_d[:, :, :])
            nc.sync.dma_start(c4t[:], c4_d[:, :, :])
            nc.sync.dma_start(bt[:], b_d[:, :])
            nc.sync.dma_start(nat[:], na_d[:, :])
            nc.sync.dma_start(epst[:], eps_d[:, :])
            nc.sync.dma_start(swt[:], sw_d[:, :])

            v = nc.vector
            load = {"dve": 0.0, "act": 0.0, "gp": 0.0}

            def emit_front(super_):
                """Loads, per-slot bn_stats, sigma'^2 (DVE TT only)."""
                k0 = super_[0][0]
                k1 = super_[-1][1]
                Mc = k1 - k0

                st = stp.tile([128, Mc, 6], F32, tag="st")
                Xs = []
                for (mk0, mk1) in super_:
                    n0 = int(offs[mk0])
                    n1 = int(offs[mk1])
                    X = xp.tile([128, 2 * (n1 - n0)], F32, tag="X")
                    nc.sync.dma_start(X[:], xt_d[:, 2 * n0:2 * n1])
                    Xs.append(X)
                    for k in range(mk0, mk1):
                        a = int(offs[k]) - n0
                        s = int(S[k])
                        nc.vector.bn_stats(st[:, k - k0, :],
                                           X[:, 2 * a:2 * (a + s)])
                        load["dve"] += (174 + 2 * s) / 0.96

                # interleaved per-(slot,half) fields, [128, 2*Mc] views:
                st_r = st[:].rearrange("p m (x y) -> p (m x) y", x=2, y=3)
                m_v = st_r[:, :, 1]          # means  (lo,hi interleaved)
                v_v = st_r[:, :, 2]          # cnt*var
                c1s = c1t[:, k0:k1, :].rearrange("p m h -> p (m h)")
                c3s = c3t[:, k0:k1, :].rearrange("p m h -> p (m h)")
                c4s = c4t[:, k0:k1, :].rearrange("p m h -> p (m h)")

                U = 2 * Mc
                mu = abp.tile([128, U], F32, tag="mu")
                q = abp.tile([128, U], F32, tag="q")
                sg = abp.tile([128, U], F32, tag="sg")

                v.tensor_tensor(mu[:], m_v, c1s, mult)          # mu
                v.tensor_tensor(q[:], m_v, m_v, mult)           # mean^2
                v.tensor_tensor(q[:], q[:], c4s, mult)          # *c4'
                v.tensor_tensor(sg[:], v_v, c3s, mult)          # cnt*var*c3'
                v.tensor_tensor(sg[:], sg[:], q[:], add)        # sigma'^2/w^2
                load["dve"] += 5 * (82 + U) / 0.96
                return [super_, Xs, mu, sg, None, None, k0]

            def emit_post(ctx):
                """sigma' = sqrt(. + EPS') on ACT (per-half bias), then
                A = reciprocal (DVE), B = -a*mu*A + b (TT + ACT affines)."""
                super_, Xs, mu, sg, _, _, k0 = ctx
                k1 = super_[-1][1]
                U = 2 * (k1 - k0)
                At = abp3.tile([128, U], F32, tag="At")
                Bt = abp3.tile([128, U], F32, tag="Bt")
                for h in (0, 1):
                    sgh = sg[:].rearrange("p (m h) -> p m h", h=2)[:, :, h]
                    nc.scalar.activation(sgh, sgh, sqrtf,
                                         bias=epst[:, h:h + 1])
                    load["act"] += (399 + 0.833 * U / 2)
                v.reciprocal(At[:], sg[:])                      # A = w*rstd
                if not w_nonneg:
                    for h in (0, 1):
                        Ah = At[:].rearrange("p (m h) -> p m h", h=2)[:, :, h]
                        v.tensor_scalar(Ah, Ah, swt[:, h:h + 1], None, mult)
                v.tensor_tensor(Bt[:], mu[:], At[:], mult)      # mu*A
                for h in (0, 1):
                    Bh = Bt[:].rearrange("p (m h) -> p m h", h=2)[:, :, h]
                    v.tensor_scalar(Bh, Bh, nat[:, h:h + 1], bt[:, h:h + 1],
                                    mult, add)
                load["dve"] += ((82 + 6 * U) + 3 * (82 + U)) / 0.96
                ctx[4] = At
                ctx[5] = Bt
                return ctx

            def emit_applies(ctx, drain=False):
                """Apply + store for a super whose A/B math was emitted
                earlier. Chunks are taken in PAIRS sharing one bf16 Y tile
                (~1 MB stores); each pair goes WHOLE to one of the three
                engines (DVE / ACT / GPSIMD) via greedy load balance --
                a Y tile shared between engines would serialize them via
                Tile deps. In the drain phase (no more fronts) DVE/ACT are
                free of stats work, so balance on a fresh ledger."""
                super_, Xs, _, _, At, Bt, k0 = ctx
                lod = drain_load if drain else load
                for pi in range(0, len(super_), 2):
                    grp = super_[pi:pi + 2]
                    gXs = Xs[pi:pi + 2]
                    gk0 = grp[0][0]
                    gk1 = grp[-1][1]
                    n0 = int(offs[gk0])
                    n1 = int(offs[gk1])
                    Y = yp.tile([128, 2 * (n1 - n0)], BF16, tag="Y")
                    Yr = Y[:].rearrange("p (w h) -> p w h", h=2)
                    costs = {
                        "dve": sum(2 * DVE_APPLY_NS(int(S[k]))
                                   for k in range(gk0, gk1)),
                        "act": sum(2 * ACT_APPLY_NS(int(S[k]))
                                   for k in range(gk0, gk1)),
                        "gp": sum(2 * GP_APPLY_NS(int(S[k]))
                                  for k in range(gk0, gk1)),
                    }
                    if not USE_GPSIMD:
                        costs.pop("gp")
                    eng = min(costs, key=lambda e: lod[e] + costs[e])
                    lod[eng] += costs[eng]
                    for mi, (mk0, mk1) in enumerate(grp):
                        mn0 = int(offs[mk0])
                        Xr = gXs[mi][:].rearrange("p (w h) -> p w h", h=2)
                        for k in range(mk0, mk1):
                            a = int(offs[k]) - mn0
                            ya = int(offs[k]) - n0
                            s = int(S[k])
                            for h in (0, 1):
                                j2 = 2 * (k - k0) + h
                                xs = Xr[:, a:a + s, h]
                                ys = Yr[:, ya:ya + s, h]
                                Ac = At[:, j2:j2 + 1]
                                Bc = Bt[:, j2:j2 + 1]
                                if eng == "dve":
                                    v.tensor_scalar(ys, xs, Ac, Bc, mult, add)
                                elif eng == "gp":
                                    nc.gpsimd.tensor_scalar(ys, xs, Ac, Bc,
                                                            mult, add)
                                else:
                                    nc.scalar.activation(ys, xs, ident,
                                                         bias=Bc, scale=Ac)
                    nc.sync.dma_start(yt_d[:, 2 * n0:2 * n1], Y[:])

            pend = []
            drain_load = {"dve": 0.0, "act": 0.0, "gp": 0.0}
            for super_ in supers:
                ctx = emit_front(super_)
                if len(pend) >= 2:
                    emit_applies(pend.pop(0))
                pend.append(emit_post(ctx))
            while pend:
                emit_applies(pend.pop(0), drain=True)
    nc.compile()
    return nc


def _build_program_cached(S, offs, supers, M, Np, w_nonneg):
    key = (tuple(int(s) for s in S), tuple(tuple(s) for s in supers), M, Np,
           w_nonneg)
    nc = _program_cache.get(key)
    if nc is None:
        nc = _build_program(S, offs, supers, M, Np, w_nonneg)
        _program_cache[key] = nc
    return nc


def kernel(x, batch, alpha, weight, bias, num_graphs):
    global _last_run
    x = np.asarray(x, dtype=np.float32)
    batch = np.asarray(batch).astype(np.int64)
    alpha = np.asarray(alpha, dtype=np.float32)
    weight = np.asarray(weight, dtype=np.float32)
    bias = np.asarray(bias, dtype=np.float32)
    G = int(num_graphs)
    N, Hx = x.shape
    assert Hx == H

    sizes = np.bincount(batch, minlength=G).astype(np.int64)
    node_order = np.argsort(batch, kind="stable")
    gstarts = np.concatenate([[0], np.cumsum(sizes)])

    ranked, S, offs, chunks = _plan_slots(sizes, N_CORES)
    M = len(S)
    Np = int(offs[-1])
    supers = _plan_supers(chunks)
    w_nonneg = bool(np.all(weight >= 0))

    nc = _build_program_cached(S, offs, supers, M, Np, w_nonneg)

    # folded constants:  sigma'^2/w^2 = (cnt*var)*c3' + bn_mean^2*c4'
    #   c3' = 1/(n*w^2); c4' = c1*(1 + caa*c1)/w^2, c1 = S/n
    #   A = w*rstd = 1/sqrt(sigma'^2/w^2 + EPS/w^2); B = b + (-a)*mu*A
    caa = alpha * alpha - 2.0 * alpha                  # [256]
    w2 = np.maximum(weight * weight, 1e-30)
    b_p = np.ascontiguousarray(bias.reshape(2, 128).T)
    na_p = np.ascontiguousarray((-alpha).reshape(2, 128).T)
    eps_p = np.ascontiguousarray((EPS / w2).reshape(2, 128).T)
    sw_p = np.ascontiguousarray(np.sign(weight).reshape(2, 128).T)

    xa = np.concatenate([x, np.zeros((1, H), np.float32)], axis=0)

    in_maps = []
    idx_per_core = []
    for c in range(N_CORES):
        gids = ranked[:, c]
        n = sizes[gids]
        idx = np.full(Np, N, dtype=np.int64)
        for k in range(M):
            g = gids[k]
            nk = int(n[k])
            if nk:
                idx[int(offs[k]):int(offs[k]) + nk] = \
                    node_order[gstarts[g]:gstarts[g] + nk]
        xp = xa[idx]                                   # [Np, 256]
        # xt[p, 2w+h] = xp[w, h*128+p]
        xv = xp.reshape(Np, 2, 128)
        xt = np.ascontiguousarray(xv.transpose(2, 0, 1)).reshape(128, 2 * Np)
        nguard = np.maximum(n, 1).astype(np.float64)
        c1 = (S.astype(np.float64) / nguard)           # [M]
        # [M, 256] per-slot-channel constants -> [128, M, 2] packed
        c3m = (1.0 / nguard)[:, None] / w2[None, :]
        c4m = (c1 * 1.0)[:, None] * (1.0 + caa[None, :] * c1[:, None]) \
            / w2[None, :]
        c1b = np.broadcast_to(c1[None, :, None], (128, M, 2)).astype(
            np.float32).copy()
        c3b = np.ascontiguousarray(
            c3m.reshape(M, 2, 128).transpose(2, 0, 1)).astype(np.float32)
        c4b = np.ascontiguousarray(
            c4m.reshape(M, 2, 128).transpose(2, 0, 1)).astype(np.float32)
        in_maps.append({
            "xt": xt, "c1": c1b, "c3": c3b, "c4": c4b,
            "bp": b_p, "nap": na_p, "epsp": eps_p, "swp": sw_p,
        })
        idx_per_core.append(idx)
    del xa

    _last_run = (nc, in_maps)
    res = run_bass_kernel_spmd(nc, in_maps, core_ids=list(range(N_CORES)))

    out = np.empty((N, H), dtype=np.float32)
    for c in range(N_CORES):
        yt = np.asarray(res.results[c]["yt"]).astype(np.float32)  # [128, 2Np]
        yv = yt.reshape(128, Np, 2)
        # out_packed[w, h*128+p] = yv[p, w, h]
        yp_ = np.ascontiguousarray(yv.transpose(1, 2, 0)).reshape(Np, H)
        idx = idx_per_core[c]
        mask = idx < N
        out[idx[mask]] = yp_[mask]
    return out


# revision 12
# speedup vs baseline: 1.0550x; 1.0550x over previous
"""GraphNorm-style segmented normalization on 8 Trainium2 NeuronCores.

Strategy (x:[500000,256] f32, batch sorted int, 4096 graphs, params [256]):

- Host: graphs sorted by size (descending), dealt round-robin to 8 cores;
  slot k on every core holds that core's rank-(8k+c) graph, padded to the
  canonical size S_k = size(rank 8k) (rounded to even). Slot structure is
  identical across cores -> one SPMD Bass program, per-core data.
- Host packs each core's nodes channel-major and HALF-INTERLEAVED:
  xt[p, 2*w + h] = x[node w, h*128 + p]. A single bn_stats over a slot's
  [128, 2*S] range yields independent stats for the lo channel half
  (even elements) and hi half (odd elements) -- one stats op per slot
  (the ISA caps BNStats at one range / 6 outputs per op).
- Device math (per slot, half h, channel c; m/v = bn mean / cnt*var):
    sigma'^2/w^2 = v*c3' + m^2*c4' (+EPS/w^2 via sqrt bias), with
    c3' = 1/(n*w^2), c4' = (S/n)*(1 + (a^2-2a)*S/n)/w^2 host constants.
    A = w*rstd = reciprocal(sqrt(.)), B = b - a*mu*A.
  This keeps only 6 TT + 1 recip per super on DVE; the small per-half
  affines (sqrt bias, B finish) ride ACT.
- Apply out = A*x + B per (slot, half): split across THREE engines (DVE
  tensor_scalar / ACT activation-Identity / GPSIMD tensor_scalar) by
  greedy cost balance, written to bf16 Y tiles (halves store traffic;
  DMA is the top bottleneck). Stores are 2-chunk (~1 MB) granules.
- Host un-interleaves, upcasts bf16 -> f32, scatters rows back.
"""
import sys

if "/opt/trn_rl_repo" not in sys.path:
    sys.path.insert(0, "/opt/trn_rl_repo")

import numpy as np

import concourse.bacc as bacc
import concourse.tile as tile
from concourse import mybir
from concourse.bass_utils import run_bass_kernel_spmd

F32 = mybir.dt.float32
BF16 = mybir.dt.bfloat16
EPS = 1e-9
N_CORES = 8
H = 256
MINI_TGT = 1024     # nodes per chunk (DMA/pipeline granule)
X_BUFS = 14         # X alive ~3 supers (applies lag fronts by 2)
Y_BUFS = 5
USE_GPSIMD = True
# measured per-op cost models (ns) for the apply split, S = slot size
DVE_APPLY_NS = lambda S: 184 + 1.042 * S
ACT_APPLY_NS = lambda S: 399 + 0.833 * S
GP_APPLY_NS = lambda S: 131 + 2.9 * S

_program_cache = {}
_last_run = None


def _plan_slots(sizes, n_cores):
    G = len(sizes)
    Gp = ((G + n_cores - 1) // n_cores) * n_cores
    sizes_p = np.concatenate([sizes, np.zeros(Gp - len(sizes), sizes.dtype)])
    order = np.argsort(-sizes_p, kind="stable")
    ranked = order.reshape(-1, n_cores)
    rank_sz = sizes_p[order].reshape(-1, n_cores)
    S = rank_sz[:, 0]
    keep = S > 0
    ranked = ranked[keep]
    S = S[keep].astype(np.int64)
    S = ((S + 1) // 2) * 2
    offs = np.concatenate([[0], np.cumsum(S)])
    M = len(S)
    chunks = []
    k0 = 0
    acc = 0
    for k in range(M):
        acc += int(S[k])
        if acc >= MINI_TGT:
            chunks.append((k0, k + 1))
            k0 = k + 1
            acc = 0
    if k0 < M:
        chunks.append((k0, M))
    return ranked, S, offs, chunks


def _plan_supers(minis):
    return [minis[i:i + 4] for i in range(0, len(minis), 4)]


def _build_program(S, offs, supers, M, Np, w_nonneg):
    nc = bacc.Bacc("TRN2", target_bir_lowering=False, debug=False,
                   num_devices=N_CORES)
    xt_d = nc.dram_tensor("xt", [128, 2 * Np], F32, kind="ExternalInput")
    c1_d = nc.dram_tensor("c1", [128, M, 2], F32, kind="ExternalInput")
    c3_d = nc.dram_tensor("c3", [128, M, 2], F32, kind="ExternalInput")
    c4_d = nc.dram_tensor("c4", [128, M, 2], F32, kind="ExternalInput")
    b_d = nc.dram_tensor("bp", [128, 2], F32, kind="ExternalInput")
    na_d = nc.dram_tensor("nap", [128, 2], F32, kind="ExternalInput")
    eps_d = nc.dram_tensor("epsp", [128, 2], F32, kind="ExternalInput")
    sw_d = nc.dram_tensor("swp", [128, 2], F32, kind="ExternalInput")
    yt_d = nc.dram_tensor("yt", [128, 2 * Np], BF16, kind="ExternalOutput")

    mult = mybir.AluOpType.mult
    add = mybir.AluOpType.add
    ident = mybir.ActivationFunctionType.Identity
    sqrtf = mybir.ActivationFunctionType.Sqrt

    with tile.TileContext(nc) as tc:
        with (
            tc.tile_pool(name="const", bufs=1) as constp,
            tc.tile_pool(name="xp", bufs=X_BUFS) as xp,
            tc.tile_pool(name="yp", bufs=Y_BUFS) as yp,
            tc.tile_pool(name="stp", bufs=2) as stp,
            tc.tile_pool(name="abp", bufs=2) as abp,
            tc.tile_pool(name="abp3", bufs=3) as abp3,
        ):
            c1t = constp.tile([128, M, 2], F32)
            c3t = constp.tile([128, M, 2], F32)
            c4t = constp.tile([128, M, 2], F32)
            bt = constp.tile([128, 2], F32)
            nat = constp.tile([128, 2], F32)
            epst = constp.tile([128, 2], F32)
            swt = constp.tile([128, 2], F32)
            nc.sync.dma_start(c1t[:], c1_d[:, :, :])
            nc.sync.dma_start(c3t[:], c3_d[:, :, :])
            nc.sync.dma_start(c4t[:], c4_d[:, :, :])
            nc.sync.dma_start(bt[:], b_d[:, :])
            nc.sync.dma_start(nat[:], na_d[:, :])
            nc.sync.dma_start(epst[:], eps_d[:, :])
            nc.sync.dma_start(swt[:], sw_d[:, :])

            v = nc.vector
            load = {"dve": 0.0, "act": 0.0, "gp": 0.0}

            def emit_front(super_):
                """Loads, per-slot bn_stats, sigma'^2 (DVE TT only)."""
                k0 = super_[0][0]
                k1 = super_[-1][1]
                Mc = k1 - k0

                st = stp.tile([128, Mc, 6], F32, tag="st")
                Xs = []
                for (mk0, mk1) in super_:
                    n0 = int(offs[mk0])
                    n1 = int(offs[mk1])
                    X = xp.tile([128, 2 * (n1 - n0)], F32, tag="X")
                    nc.sync.dma_start(X[:], xt_d[:, 2 * n0:2 * n1])
                    Xs.append(X)
                    for k in range(mk0, mk1):
                        a = int(offs[k]) - n0
                        s = int(S[k])
                        nc.vector.bn_stats(st[:, k - k0, :],
                                           X[:, 2 * a:2 * (a + s)])
                        load["dve"] += (174 + 2 * s) / 0.96

                # interleaved per-(slot,half) fields, [128, 2*Mc] views:
                st_r = st[:].rearrange("p m (x y) -> p (m x) y", x=2, y=3)
                m_v = st_r[:, :, 1]          # means  (lo,hi interleaved)
                v_v = st_r[:, :, 2]          # cnt*var
                c1s = c1t[:, k0:k1, :].rearrange("p m h -> p (m h)")
                c3s = c3t[:, k0:k1, :].rearrange("p m h -> p (m h)")
                c4s = c4t[:, k0:k1, :].rearrange("p m h -> p (m h)")

                U = 2 * Mc
                mu = abp.tile([128, U], F32, tag="mu")
                q = abp.tile([128, U], F32, tag="q")
                sg = abp.tile([128, U], F32, tag="sg")

                v.tensor_tensor(mu[:], m_v, c1s, mult)          # mu
                v.tensor_tensor(q[:], m_v, m_v, mult)           # mean^2
                v.tensor_tensor(q[:], q[:], c4s, mult)          # *c4'
                v.tensor_tensor(sg[:], v_v, c3s, mult)          # cnt*var*c3'
                v.tensor_tensor(sg[:], sg[:], q[:], add)        # sigma'^2/w^2
                load["dve"] += 5 * (82 + U) / 0.96
                return [super_, Xs, mu, sg, None, None, k0]

            def emit_post(ctx):
                """sigma' = sqrt(. + EPS') on ACT (per-half bias), then
                A = reciprocal (DVE), B = -a*mu*A + b (TT + ACT affines)."""
                super_, Xs, mu, sg, _, _, k0 = ctx
                k1 = super_[-1][1]
                U = 2 * (k1 - k0)
                At = abp3.tile([128, U], F32, tag="At")
                Bt = abp3.tile([128, U], F32, tag="Bt")
                for h in (0, 1):
                    sgh = sg[:].rearrange("p (m h) -> p m h", h=2)[:, :, h]
                    nc.scalar.activation(sgh, sgh, sqrtf,
                                         bias=epst[:, h:h + 1])
                    load["act"] += (399 + 0.833 * U / 2)
                v.reciprocal(At[:], sg[:])                      # A = w*rstd
                if not w_nonneg:
                    for h in (0, 1):
                        Ah = At[:].rearrange("p (m h) -> p m h", h=2)[:, :, h]
                        v.tensor_scalar(Ah, Ah, swt[:, h:h + 1], None, mult)
                v.tensor_tensor(Bt[:], mu[:], At[:], mult)      # mu*A
                for h in (0, 1):
                    Bh = Bt[:].rearrange("p (m h) -> p m h", h=2)[:, :, h]
                    v.tensor_scalar(Bh, Bh, nat[:, h:h + 1], bt[:, h:h + 1],
                                    mult, add)
                load["dve"] += ((82 + 6 * U) + 3 * (82 + U)) / 0.96
                ctx[4] = At
                ctx[5] = Bt
                return ctx

            def emit_applies(ctx, drain=False):
                """Apply + store for a super whose A/B math was emitted
                earlier. Chunks are taken in PAIRS sharing one bf16 Y tile
                (~1 MB stores); each pair goes WHOLE to one of the three
                engines (DVE / ACT / GPSIMD) via greedy load balance --
                a Y tile shared between engines would serialize them via
                Tile deps. In the drain phase (no more fronts) DVE/ACT are
                free of stats work, so balance on a fresh ledger."""
                super_, Xs, _, _, At, Bt, k0 = ctx
                lod = drain_load if drain else load
                for pi in range(0, len(super_), 2):
                    grp = super_[pi:pi + 2]
                    gXs = Xs[pi:pi + 2]
                    gk0 = grp[0][0]
                    gk1 = grp[-1][1]
                    n0 = int(offs[gk0])
                    n1 = int(offs[gk1])
                    Y = yp.tile([128, 2 * (n1 - n0)], BF16, tag="Y")
                    Yr = Y[:].rearrange("p (w h) -> p w h", h=2)
                    costs = {
                        "dve": sum(2 * DVE_APPLY_NS(int(S[k]))
                                   for k in range(gk0, gk1)),
                        "act": sum(2 * ACT_APPLY_NS(int(S[k]))
                                   for k in range(gk0, gk1)),
                        "gp": sum(2 * GP_APPLY_NS(int(S[k]))
                                  for k in range(gk0, gk1)),
                    }
                    if not USE_GPSIMD:
                        costs.pop("gp")
                    eng = min(costs, key=lambda e: lod[e] + costs[e])
                    lod[eng] += costs[eng]
                    for mi, (mk0, mk1) in enumerate(grp):
                        mn0 = int(offs[mk0])
                        Xr = gXs[mi][:].rearrange("p (w h) -> p w h", h=2)
                        for k in range(mk0, mk1):
                            a = int(offs[k]) - mn0
                            ya = int(offs[k]) - n0
                            s = int(S[k])
                            for h in (0, 1):
                                j2 = 2 * (k - k0) + h
                                xs = Xr[:, a:a + s, h]
                                ys = Yr[:, ya:ya + s, h]
                                Ac = At[:, j2:j2 + 1]
                                Bc = Bt[:, j2:j2 + 1]
                                if eng == "dve":
                                    v.tensor_scalar(ys, xs, Ac, Bc, mult, add)
                                elif eng == "gp":
                                    nc.gpsimd.tensor_scalar(ys, xs, Ac, Bc,
                                                            mult, add)
                                else:
                                    nc.scalar.activation(ys, xs, ident,
                                                         bias=Bc, scale=Ac)
                    nc.sync.dma_start(yt_d[:, 2 * n0:2 * n1], Y[:])

            # Pipeline: front(s) | post(s-1) | applies(s-2). The one-super
            # post lag guarantees the ACT sqrt's input (DVE sg) finished a
            # whole super earlier, so the sqrt at ACT's queue head never
            # stalls ACT (and DVE's recip behind it never convoys).
            pend = []
            done = []
            drain_load = {"dve": 0.0, "act": 0.0, "gp": 0.0}
            for super_ in supers:
                ctx = emit_front(super_)
                if pend:
                    done.append(emit_post(pend.pop(0)))
                if len(done) >= 2:
                    emit_applies(done.pop(0))
                pend.append(ctx)
            while pend:
                done.append(emit_post(pend.pop(0)))
            while done:
                emit_applies(done.pop(0), drain=True)
    nc.compile()
    return nc


def _build_program_cached(S, offs, supers, M, Np, w_nonneg):
    key = (tuple(int(s) for s in S), tuple(tuple(s) for s in supers), M, Np,
           w_nonneg)
    nc = _program_cache.get(key)
    if nc is None:
        nc = _build_program(S, offs, supers, M, Np, w_nonneg)
        _program_cache[key] = nc
    return nc


def kernel(x, batch, alpha, weight, bias, num_graphs):
    global _last_run
    x = np.asarray(x, dtype=np.float32)
    batch = np.asarray(batch).astype(np.int64)
    alpha = np.asarray(alpha, dtype=np.float32)
    weight = np.asarray(weight, dtype=np.float32)
    bias = np.asarray(bias, dtype=np.float32)
    G = int(num_graphs)
    N, Hx = x.shape
    assert Hx == H

    sizes = np.bincount(batch, minlength=G).astype(np.int64)
    node_order = np.argsort(batch, kind="stable")
    gstarts = np.concatenate([[0], np.cumsum(sizes)])

    ranked, S, offs, chunks = _plan_slots(sizes, N_CORES)
    M = len(S)
    Np = int(offs[-1])
    supers = _plan_supers(chunks)
    w_nonneg = bool(np.all(weight >= 0))

    nc = _build_program_cached(S, offs, supers, M, Np, w_nonneg)

    # folded constants:  sigma'^2/w^2 = (cnt*var)*c3' + bn_mean^2*c4'
    #   c3' = 1/(n*w^2); c4' = c1*(1 + caa*c1)/w^2, c1 = S/n
    #   A = w*rstd = 1/sqrt(sigma'^2/w^2 + EPS/w^2); B = b + (-a)*mu*A
    caa = alpha * alpha - 2.0 * alpha                  # [256]
    w2 = np.maximum(weight * weight, 1e-30)
    b_p = np.ascontiguousarray(bias.reshape(2, 128).T)
    na_p = np.ascontiguousarray((-alpha).reshape(2, 128).T)
    eps_p = np.ascontiguousarray((EPS / w2).reshape(2, 128).T)
    sw_p = np.ascontiguousarray(np.sign(weight).reshape(2, 128).T)

    xa = np.concatenate([x, np.zeros((1, H), np.float32)], axis=0)

    in_maps = []
    idx_per_core = []
    for c in range(N_CORES):
        gids = ranked[:, c]
        n = sizes[gids]
        idx = np.full(Np, N, dtype=np.int64)
        for k in range(M):
            g = gids[k]
            nk = int(n[k])
            if nk:
                idx[int(offs[k]):int(offs[k]) + nk] = \
                    node_order[gstarts[g]:gstarts[g] + nk]
        xp = xa[idx]                                   # [Np, 256]
        # xt[p, 2w+h] = xp[w, h*128+p]
        xv = xp.reshape(Np, 2, 128)
        xt = np.ascontiguousarray(xv.transpose(2, 0, 1)).reshape(128, 2 * Np)
        nguard = np.maximum(n, 1).astype(np.float64)
        c1 = (S.astype(np.float64) / nguard)           # [M]
        # [M, 256] per-slot-channel constants -> [128, M, 2] packed
        c3m = (1.0 / nguard)[:, None] / w2[None, :]
        c4m = (c1 * 1.0)[:, None] * (1.0 + caa[None, :] * c1[:, None]) \
            / w2[None, :]
        c1b = np.broadcast_to(c1[None, :, None], (128, M, 2)).astype(
            np.float32).copy()
        c3b = np.ascontiguousarray(
            c3m.reshape(M, 2, 128).transpose(2, 0, 1)).astype(np.float32)
        c4b = np.ascontiguousarray(
            c4m.reshape(M, 2, 128).transpose(2, 0, 1)).astype(np.float32)
        in_maps.append({
            "xt": xt, "c1": c1b, "c3": c3b, "c4": c4b,
            "bp": b_p, "nap": na_p, "epsp": eps_p, "swp": sw_p,
        })
        idx_per_core.append(idx)
    del xa

    _last_run = (nc, in_maps)
    res = run_bass_kernel_spmd(nc, in_maps, core_ids=list(range(N_CORES)))

    out = np.empty((N, H), dtype=np.float32)
    for c in range(N_CORES):
        yt = np.asarray(res.results[c]["yt"]).astype(np.float32)  # [128, 2Np]
        yv = yt.reshape(128, Np, 2)
        # out_packed[w, h*128+p] = yv[p, w, h]
        yp_ = np.ascontiguousarray(yv.transpose(1, 2, 0)).reshape(Np, H)
        idx = idx_per_core[c]
        mask = idx < N
        out[idx[mask]] = yp_[mask]
    return out
